# revision 12
# baseline (speedup 1.0000x reference)
"""Mixture-of-Experts Trainium2 kernel (8-core SPMD, token-sharded, bf16).

v6: head restructure guided by the NTFF traces of v4 (470.2us) and the
failed v5 (478.9us):
  * v4's PE only went dense at ~25us: the gpsimd queue ran ahead and
    issued 8MB of next-expert prefetches whose SDMA packets round-robin-
    stole HBM bandwidth from the critical xt loads.
  * v5 split w1_0 into 16 small SWDGE DMAs -- issue-rate limited (~0.9us
    per issue, stretching to 3us+ under descriptor-ring backlog), which
    starved L1 even harder.
  * v6: 4 xt tiles [P,DC,512] and 4 w1_0 fc-group tiles [P,4,DC*128],
    each host-packed so a tile is 4KB-contiguous per partition (128 fat
    descriptors per DMA).  Critical tiles go first on each ring (sync:
    xt0,xt2; scalar: xt1,xt3; gpsimd: w1g0-3); big prefetches are emitted
    behind them so ring FIFO keeps the critical window clean.  gw/b1 ride
    the otherwise-idle vector queue.  Warmup extended to 96 matmuls to
    keep the HAM clock-gate warm until the first real matmul (~10.5us).
  * tail kept from v4 (measured at its floor: ~0.75us combine + 0.6us
    issue + ~3us DMA completion receipt); output DMAs alternate rings.
"""

import numpy as np
import ml_dtypes
from contextlib import ExitStack

import bass_rust as _bass_rust
import concourse.bass as bass
import concourse.mybir as mybir
import concourse.tile as tile
from concourse.bass_utils import run_bass_kernel_spmd

BF16 = mybir.dt.bfloat16
F32 = mybir.dt.float32
N_CORES = 8
P = 128


# ---------------------------------------------------------------------------
# Workaround for walrus "Too many sync wait commands" (see baseline).
# ---------------------------------------------------------------------------
_split_ctr = [0]


def _split_multi_waits(nc):
    for f in nc.m.functions:
        for blk in f.blocks:
            insts = blk.instructions
            i = 0
            while i < len(insts):
                inst = insts[i]
                si = getattr(inst, "sync_info", None)
                waits = list(si.on_wait) if si is not None and si.on_wait else []
                if len(waits) > 1:
                    si.on_wait = waits[-1:]
                    for w in waits[:-1]:
                        _split_ctr[0] += 1
                        ev = mybir.InstEventSemaphore(
                            name=f"I-wsplit-{_split_ctr[0]}", ins=[], outs=[]
                        )
                        ev.engine = inst.engine
                        ev.sync_info = _bass_rust.SyncInfo(
                            on_wait=[w], on_update=[]
                        )
                        insts.insert(i, ev)
                        i += 1
                i += 1


# ---------------------------------------------------------------------------
# Device kernel
# ---------------------------------------------------------------------------
def build_moe_kernel(K: int, T: int, D: int, DF: int):
    assert T % 512 == 0 and D % P == 0 and DF % P == 0
    TT = T // P       # 128-token tiles
    TC = T // 512     # 512-token chunks
    DC = D // P       # D chunks of 128
    FC = DF // P      # F chunks of 128
    FG = 4            # fc per w1_0 head tile
    NG = FC // FG

    nc = bass.Bass("TRN2", target_bir_lowering=False)

    # x packed [P, TC, DC, 512]: xq[p, tcc, dc, j] = x[tcc*512+j, dc*128+p]
    xq = nc.declare_dram_parameter("xq", [P, TC, DC, 512], BF16, isOutput=False)
    # expert-0 w1, fc-major pack: w1z[p, fc, dc*128+j] = w1[0][dc*128+p, fc*128+j]
    w1z = nc.declare_dram_parameter("w1z", [P, FC, DC * P], BF16, isOutput=False)
    # packed: w1s[k, p, dc, f] = w1[k, dc*128 + p, f]   (used for k >= 1)
    w1s = nc.declare_dram_parameter("w1s", [K, P, DC, DF], BF16, isOutput=False)
    # packed: w2s[k, p, fc, d] = w2[k, fc*128 + p, d]
    w2s = nc.declare_dram_parameter("w2s", [K, P, FC, D], BF16, isOutput=False)
    # gwp[p, tt*K + k] = softmax gate weight for token tt*128+p, expert k
    gwp = nc.declare_dram_parameter("gwp", [P, TT * K], F32, isOutput=False)
    # b1pk[p, k*FC + fc] = b1[k, fc*128 + p]
    b1pk = nc.declare_dram_parameter("b1pk", [P, K * FC], F32, isOutput=False)
    out = nc.declare_dram_parameter("out", [T, D], BF16, isOutput=True)

    mult = mybir.AluOpType.mult
    add = mybir.AluOpType.add
    gelu_fn = mybir.ActivationFunctionType.Gelu_apprx_tanh

    with tile.TileContext(nc) as tc:
        with ExitStack() as ctx:
            persist = ctx.enter_context(tc.tile_pool(name="persist", bufs=1))
            w1gp = ctx.enter_context(tc.tile_pool(name="w1gp", bufs=NG))
            w1p = ctx.enter_context(tc.tile_pool(name="w1p", bufs=2))
            w2p = ctx.enter_context(tc.tile_pool(name="w2p", bufs=2))
            hp = ctx.enter_context(tc.tile_pool(name="hp", bufs=FC))
            ob = ctx.enter_context(tc.tile_pool(name="ob", bufs=4))
            psA = ctx.enter_context(tc.tile_pool(name="psA", bufs=4, space="PSUM"))
            psB = ctx.enter_context(tc.tile_pool(name="psB", bufs=3, space="PSUM"))

            # ---- critical-path loads, critical-first per ring ----
            # w1_0 as NG fc-group tiles on the gpsimd ring; x as TC per-tcc
            # tiles split across the two HWDGE rings; gw/b1 on the vector
            # queue so they don't delay any critical ring.
            w1g = [
                w1gp.tile([P, FG, DC * P], BF16, tag="w1g", name=f"w1g_{g}")
                for g in range(NG)
            ]
            xt = [
                persist.tile([P, DC, 512], BF16, tag=f"xt{tcc}", name=f"xt{tcc}")
                for tcc in range(TC)
            ]
            b1_sb = persist.tile([P, K * FC], F32, tag="b1", name="b1_sb")
            gw_sb = persist.tile([P, TT * K], F32, tag="gw", name="gw_sb")
            # Criticality order: first matmul needs only w1g0 + xt0; the
            # rest of fc0 needs xt1-3; w1g1-3 have 3.4us/fc of slack.  The
            # tiny b1 (first gelu) and gw (first combine, ~65us) ride the
            # gpsimd path whose transfers share bandwidth concurrently.
            # Ring FIFO is the only priority mechanism: xt0 and w1g0 sit
            # alone at the head of their rings so the first matmul's 1MB
            # critical set transfers at full bandwidth.  xt1/xt2 queue
            # behind xt0 on sync; xt3 is issued from the gelu-throttled
            # scalar queue at a group milestone (see _milestones).
            nc.gpsimd.dma_start(w1g[0][:], w1z[:, 0:FG, :])
            nc.sync.dma_start(xt[0][:], xq[:, 0])
            nc.gpsimd.dma_start(b1_sb[:], b1pk[:])
            for tcc in range(1, min(TC, 3)):
                nc.sync.dma_start(xt[tcc][:], xq[:, tcc])
            for g in range(1, NG):
                nc.gpsimd.dma_start(w1g[g][:], w1z[:, g * FG:(g + 1) * FG, :])
            nc.gpsimd.dma_start(gw_sb[:], gwp[:])

            acc = [
                persist.tile([P, D], F32, tag=f"acc{t}", name=f"acc{t}")
                for t in range(TT)
            ]

            # ---- PE + ACT warmup during the DMA head ----
            # The PE runs ~2x slower until ~3.4us of sustained activity
            # (HAM clock gate); dummy matmuls on a zeroed scratch tile keep
            # it busy while the critical DMAs land.  A dummy gelu
            # pre-triggers the ~1.3us ACT_TABLE_LOAD for the gelu set.
            warm = persist.tile([P, 16], BF16, tag="warm", name="warm")
            nc.vector.memset(warm[:], 0)
            warm_ps = psB.tile([P, 512], F32, tag="po", name="warm_ps")
            for r in range(110):
                nc.tensor.matmul(
                    warm_ps[0:16, 0:16], warm[:], warm[:],
                    start=True, stop=True,
                )
            warm_h = persist.tile([P, 16], BF16, tag="warmh", name="warm_h")
            nc.scalar.activation(warm_h[:], warm[:], gelu_fn)

            def emit_l1(k, stat, after_group=None, quad=False):
                """h[F,T] = gelu(W1.T @ x + b1); stat(dc, fc) -> stationary AP.

                dc innermost: each psum group completes in 4 consecutive
                matmuls and its gelu issues immediately -- smooth ACT cadence.
                quad=True (expert 0) iterates fc-quadrant-major so the first
                FG groups need only w1g0 + xt0, and each later input tile's
                deadline is a full quadrant (~13.6us) out.
                """
                ht = [
                    hp.tile([P, T], BF16, tag="h", name=f"h_{k}_{fc}")
                    for fc in range(FC)
                ]
                if quad:
                    order = [
                        (fc, tcc)
                        for g in range(NG)
                        for tcc in range(TC)
                        for fc in range(g * FG, (g + 1) * FG)
                    ]
                else:
                    order = [
                        (fc, tcc) for fc in range(FC) for tcc in range(TC)
                    ]
                for gidx, (fc, tcc) in enumerate(order):
                    ph = psA.tile(
                        [P, 512], F32, tag="ph", name=f"ph_{k}_{fc}_{tcc}"
                    )
                    for dc in range(DC):
                        nc.tensor.matmul(
                            ph[:],
                            stat(dc, fc),
                            xt[tcc][:, dc, :],
                            start=(dc == 0),
                            stop=(dc == DC - 1),
                        )
                    nc.scalar.activation(
                        ht[fc][:, tcc * 512:(tcc + 1) * 512], ph[:], gelu_fn,
                        bias=b1_sb[:, k * FC + fc:k * FC + fc + 1],
                    )
                    if after_group is not None:
                        after_group(gidx + 1)
                return ht

            def emit_l2(k, ht, w2t):
                """eo[T,D] = h.T @ W2 ; acc (+)= eo * gw[:,k]; store when k==K-1."""
                for tt in range(TT):
                    po = psB.tile([P, 512], F32, tag="po", name=f"po_{k}_{tt}")
                    for fc in range(FC):
                        nc.tensor.matmul(
                            po[:, 0:D],
                            ht[fc][:, tt * P:(tt + 1) * P],
                            w2t[:, fc, :],
                            start=(fc == 0),
                            stop=(fc == FC - 1),
                        )
                    g = gw_sb[:, tt * K + k:tt * K + k + 1]
                    if k == K - 1 and K == 1:
                        o = ob.tile([P, D], BF16, tag="o", name=f"o_{tt}")
                        nc.vector.tensor_scalar_mul(o[:], po[:, 0:D], g)
                        eng = nc.sync if tt % 2 == 0 else nc.scalar
                        eng.dma_start(out[tt * P:(tt + 1) * P, :], o[:])
                    elif k == 0:
                        nc.vector.tensor_scalar_mul(acc[tt][:], po[:, 0:D], g)
                    elif k < K - 1:
                        nc.vector.scalar_tensor_tensor(
                            acc[tt][:], po[:, 0:D], g,
                            acc[tt][:], op0=mult, op1=add,
                        )
                    else:
                        o = ob.tile([P, D], BF16, tag="o", name=f"o_{tt}")
                        nc.vector.scalar_tensor_tensor(
                            o[:], po[:, 0:D], g,
                            acc[tt][:], op0=mult, op1=add,
                        )
                        eng = nc.sync if tt % 2 == 0 else nc.scalar
                        eng.dma_start(out[tt * P:(tt + 1) * P, :], o[:])

            # ---- weight tiles + prefetch scheduling ----
            # Queue-program placement does NOT throttle a DMA issue -- an
            # idle queue runs ahead (v4/v6 lesson: early 2MB prefetches
            # stole HBM bandwidth from the critical head loads).  The only
            # queue whose issues are compute-throttled is scalar (gelu
            # cadence), so the four prefetches with no pool-WAR guard
            # (w2_0, w1_1, w2_1, w1_2) issue there at fc milestones.  The
            # rest (w2_2+, w1_3+) reuse pool buffers, so their WAR
            # semaphores already hold them until mid-kernel; they ride the
            # otherwise-idle gpsimd queue.
            w2t = [
                w2p.tile([P, FC, D], BF16, tag="w2", name=f"w2_{k}")
                for k in range(K)
            ]
            w1t = [None] + [
                w1p.tile([P, DC, DF], BF16, tag="w1", name=f"w1_{k}")
                for k in range(1, K)
            ]

            quadlen = FG * TC
            _milestones = {}
            if TC > 3:
                _milestones[FG] = lambda: nc.scalar.dma_start(
                    xt[3][:], xq[:, 3])
            _milestones[2 * FG] = lambda: nc.scalar.dma_start(
                w2t[0][:], w2s[0])
            if K > 1:
                _milestones[quadlen + FG] = lambda: nc.scalar.dma_start(
                    w1t[1][:], w1s[1])
                _milestones[2 * quadlen + FG] = lambda: nc.scalar.dma_start(
                    w2t[1][:], w2s[1])
            if K > 2:
                _milestones[3 * quadlen + FG] = lambda: nc.scalar.dma_start(
                    w1t[2][:], w1s[2])

            def _prefetch0(gidx):
                act = _milestones.get(gidx)
                if act is not None:
                    act()

            ht = emit_l1(
                0,
                lambda dc, fc: w1g[fc // FG][:, fc % FG, dc * P:(dc + 1) * P],
                after_group=_prefetch0,
                quad=True,
            )
            emit_l2(0, ht, w2t[0])

            for k in range(1, K):
                if k >= 2:
                    nc.gpsimd.dma_start(w2t[k][:], w2s[k])
                if k + 1 < K and k + 1 >= 3:
                    nc.gpsimd.dma_start(w1t[k + 1][:], w1s[k + 1])
                ht = emit_l1(
                    k,
                    lambda dc, fc, w=w1t[k]: w[:, dc, fc * P:(fc + 1) * P],
                )
                emit_l2(k, ht, w2t[k])

    _split_multi_waits(nc)
    return nc


# ---------------------------------------------------------------------------
# Host wrapper
# ---------------------------------------------------------------------------
_NC_CACHE: dict = {}


def _get_nc(K: int, T: int, D: int, DF: int):
    key = (K, T, D, DF)
    if key not in _NC_CACHE:
        _NC_CACHE[key] = build_moe_kernel(K, T, D, DF)
    return _NC_CACHE[key]


def _softmax(x, axis=-1):
    m = np.max(x, axis=axis, keepdims=True)
    e = np.exp(x - m)
    return e / np.sum(e, axis=axis, keepdims=True)


def run(inputs: dict, trace: bool = False, tmpdir: str | None = None):
    x = np.asarray(inputs["x"], dtype=np.float32)
    gate_w = np.asarray(inputs["gate_w"], dtype=np.float32)
    gate_b = np.asarray(inputs["gate_b"], dtype=np.float32)
    w1 = np.asarray(inputs["w1"], dtype=np.float32)
    b1 = np.asarray(inputs["b1"], dtype=np.float32)
    w2 = np.asarray(inputs["w2"], dtype=np.float32)
    b2 = np.asarray(inputs["b2"], dtype=np.float32)
    K = int(inputs["num_available"])

    B, S, D = x.shape
    DF = w1.shape[2]
    Ttot = B * S
    T = Ttot // N_CORES
    DC = D // P
    FC = DF // P
    TT = T // P
    TC = T // 512

    # Coarse routing on host (tiny): gate applied to the global token sum.
    ksum = x.sum(axis=(0, 1))
    coarse = gate_w @ ksum + gate_b
    idx = np.argsort(-coarse, kind="stable")[:K]

    gws = gate_w[idx]                      # [K, D]
    gbs = gate_b[idx]                      # [K]
    b1s = np.ascontiguousarray(b1[idx], dtype=np.float32)              # [K,DF]
    b2s = np.ascontiguousarray(b2[idx], dtype=np.float32)              # [K,D]

    # per-token softmax gating on host (0.02% of the FLOPs)
    xf = x.reshape(Ttot, D)
    logits = xf @ gws.T + gbs[None, :]                                 # [Ttot,K]
    gw = _softmax(logits, axis=1).astype(np.float32)

    # packed weights
    w1sel = np.ascontiguousarray(
        w1[idx].reshape(K, DC, P, DF).transpose(0, 2, 1, 3)
    ).astype(ml_dtypes.bfloat16)                                       # [K,P,DC,DF]
    w2sel = np.ascontiguousarray(
        w2[idx].reshape(K, FC, P, D).transpose(0, 2, 1, 3)
    ).astype(ml_dtypes.bfloat16)                                       # [K,P,FC,D]
    # expert-0 w1, fc-major pack: [P, FC, DC*128]
    w1z = np.ascontiguousarray(
        w1[idx[0]].reshape(DC, P, FC, P).transpose(1, 2, 0, 3).reshape(P, FC, DC * P)
    ).astype(ml_dtypes.bfloat16)
    b1pk = np.ascontiguousarray(
        b1s.reshape(K, FC, P).transpose(2, 0, 1).reshape(P, K * FC), dtype=np.float32
    )

    nc = _get_nc(K, T, D, DF)
    in_maps = []
    for c in range(N_CORES):
        gwc = gw[c * T:(c + 1) * T]  # [T, K]
        gwp = np.ascontiguousarray(
            gwc.reshape(TT, P, K).transpose(1, 0, 2).reshape(P, TT * K),
            dtype=np.float32,
        )
        # x packed [P, TC, DC, 512]
        xc = xf[c * T:(c + 1) * T]
        xqc = np.ascontiguousarray(
            xc.reshape(TC, 512, DC, P).transpose(3, 0, 2, 1)
        ).astype(ml_dtypes.bfloat16)
        in_maps.append({
            "xq": xqc,
            "w1z": w1z,
            "w1s": w1sel,
            "w2s": w2sel,
            "gwp": gwp,
            "b1pk": b1pk,
        })

    res = run_bass_kernel_spmd(
        nc, in_maps, list(range(N_CORES)), trace=trace, tmpdir=tmpdir
    )
    outp = np.concatenate(
        [np.asarray(res.results[c]["out"]) for c in range(N_CORES)], axis=0
    ).astype(np.float32).reshape(B, S, D)

    # b2 contribution (zero in this problem's inputs; exact host-side fallback)
    if np.any(b2s):
        outp = outp + (gw @ b2s).reshape(B, S, D)

    return outp, res


def kernel(**inputs) -> np.ndarray:
    outp, _ = run(inputs, trace=False)
    return outp
